# revision 15
# baseline (speedup 1.0000x reference)
"""Trainium2 Bass kernel for nn_ArgmaxBase_46523085750826.

Reference computation (per row b of a batch B):
  decimal = sum_k category_factors[k] * inputs_categorical[b, k]        (int)
  bit_j   = (decimal & binary_mask[j]) != 0,  binary_mask[j] = 2^(20-j)
  v0_j = noise[b, 2j], v1_j = noise[b, 2j+1]
  out_noise[b, 2j]   = bit_j ? v0_j      : v0_j*v1_j
  out_noise[b, 2j+1] = bit_j ? v0_j*v1_j : v1_j
  out_continuous[b]  = concat(inputs_continuous[b], out_noise[b])       [106]
  log_probs[b]       = sum_j log(bit_j ? v0_j : v1_j)

category_factors = [6^0 .. 6^7] (mixed radix 6) and binary_mask =
[2^20 .. 2^0] are structural constants of the problem (NCAT=6, NB=21)
and are folded into the kernel (Horner encode + per-column shifts).

Pure data parallel across 8 NeuronCores: core i handles rows
[i*B/8, (i+1)*B/8).
"""

import sys
import numpy as np

if "/opt/trn_rl_repo" not in sys.path:
    sys.path.insert(0, "/opt/trn_rl_repo")

from concourse import bacc, bass, mybir, tile
from concourse.bass_utils import run_bass_kernel_spmd

B = 524288
C = 64          # continuous dims
D = 8           # categorical dims
NCAT = 6
NB = 21         # binary dims
NOUT = C + 2 * NB  # 106
N_CORES = 8
R = B // N_CORES   # rows per core = 65536
P = 128            # partitions


def _stt_int(nc, out, in0, imm, in1, op0, op1):
    """scalar_tensor_tensor with an int32 immediate: out = (in0 op0 imm) op1 in1."""
    eng = nc.vector
    return eng.add_instruction(
        mybir.InstTensorScalarPtr(
            name=nc.get_next_instruction_name(),
            is_scalar_tensor_tensor=True,
            op0=op0, op1=op1,
            ins=[eng.lower_ap(in0),
                 mybir.ImmediateValue(dtype=mybir.dt.int32, value=imm),
                 eng.lower_ap(in1)],
            outs=[eng.lower_ap(out)],
        ))


def build_nc(rows: int = R, G: int = 64, bufs: int = 2):
    """Build the per-core Bass graph for a shard of `rows` rows.

    Row -> (partition, slot) mapping is blocked: partition p owns rows
    [p*rows/128, (p+1)*rows/128); each tile covers G consecutive slots
    per partition (128*G rows).
    """
    n_per_p = rows // P
    assert rows % P == 0 and n_per_p % G == 0
    n_tiles = n_per_p // G

    f32 = mybir.dt.float32
    i32 = mybir.dt.int32

    # Bacc (not plain Bass): its compile() runs generate_event_semaphores,
    # which splits multi-sem waits down to the 1-embedded-wait-per-
    # instruction limit of the TRN2 instruction encodings.
    nc = bacc.Bacc()
    cont_d = nc.declare_dram_parameter("inputs_continuous", [rows, C], f32, isOutput=False)
    cat_d = nc.declare_dram_parameter("inputs_categorical", [rows, D], i32, isOutput=False)
    noise_d = nc.declare_dram_parameter("dequantization_noise", [rows, 2 * NB], f32, isOutput=False)
    out_d = nc.declare_dram_parameter("out_continuous", [rows, NOUT], f32, isOutput=True)
    lp_d = nc.declare_dram_parameter("log_probs", [rows], f32, isOutput=True)

    cont_v = cont_d.rearrange("(p n) c -> p n c", p=P)
    cat_v = cat_d.rearrange("(p n) c -> p n c", p=P)
    noise_v = noise_d.rearrange("(p n) c -> p n c", p=P)
    out_v = out_d.rearrange("(p n) c -> p n c", p=P)
    lp_v = lp_d.rearrange("(p n) -> p n", p=P)

    with tile.TileContext(nc) as tc:
        with tc.tile_pool(name="consts", bufs=1) as cpool, \
             tc.tile_pool(name="lp", bufs=max(2, rows // (P * G))) as lpool, \
             tc.tile_pool(name="work", bufs=bufs) as pool:
            # shift amounts per binary column: 20, 19, ..., 0. Built with DVE
            # memsets (not gpsimd iota) so consumers carry no cross-engine
            # wait: TT/STT encodings only fit one embedded sync wait.
            sh_t = cpool.tile([P, NB], i32)
            for j in range(NB):
                nc.vector.memset(sh_t[:, j:j + 1], NB - 1 - j)

            for t in range(n_tiles):
                sl = slice(t * G, (t + 1) * G)
                cat_t = pool.tile([P, G, D], i32)
                noise_t = pool.tile([P, G, 2 * NB], f32)
                out_t = pool.tile([P, G, NOUT], f32)
                nc.sync.dma_start(out=cat_t[:, :, :], in_=cat_v[:, sl, :])
                nc.sync.dma_start(out=noise_t[:, :, :], in_=noise_v[:, sl, :])
                nc.sync.dma_start(out=out_t[:, :, 0:C], in_=cont_v[:, sl, :])

                # decimal encode via Horner: dec = (..(c7*6 + c6)*6 ..)*6 + c0
                dec_t = pool.tile([P, G], i32)
                nc.vector.tensor_copy(out=dec_t[:, :], in_=cat_t[:, :, D - 1])
                for k in range(D - 2, -1, -1):
                    nc.vector.scalar_tensor_tensor(
                        out=dec_t[:, :], in0=dec_t[:, :], scalar=float(NCAT),
                        in1=cat_t[:, :, k],
                        op0=mybir.AluOpType.mult, op1=mybir.AluOpType.add)

                # bits[p,g,j] = (dec[p,g] >> (20-j)) & 1  -> uint8 0/1
                # scalar_tensor_tensor (TSP encoding) instead of tensor_tensor:
                # the TT encoding only fits one embedded sync wait, which the
                # tile scheduler can exceed here. Integer immediates must be
                # emitted manually (the helper always lowers them as f32).
                shift_t = pool.tile([P, G, NB], i32)
                _stt_int(nc, out=shift_t[:, :, :],
                         in0=dec_t[:, :, None].broadcast_to([P, G, NB]),
                         imm=0,
                         in1=sh_t[:, None, :].broadcast_to([P, G, NB]),
                         op0=mybir.AluOpType.bypass,
                         op1=mybir.AluOpType.logical_shift_right)
                # bits = (shift & 1) `bypass` shift  (in1 unused by bypass)
                bits_t = pool.tile([P, G, NB], i32)
                _stt_int(nc, out=bits_t[:, :, :], in0=shift_t[:, :, :],
                         imm=1, in1=shift_t[:, :, :],
                         op0=mybir.AluOpType.bitwise_and,
                         op1=mybir.AluOpType.bypass)

                v0 = noise_t[:, :, 0:2 * NB:2]
                v1 = noise_t[:, :, 1:2 * NB:2]

                prod_t = pool.tile([P, G, NB], f32)
                nc.vector.scalar_tensor_tensor(
                    out=prod_t[:, :, :], in0=v0, scalar=0.0, in1=v1,
                    op0=mybir.AluOpType.bypass, op1=mybir.AluOpType.mult)

                # n0 = bit ? v0 : prod ; n1 = bit ? prod : v1
                # (computed in contiguous tiles; interleaved into out_t after)
                n0_t = pool.tile([P, G, NB], f32)
                n1_t = pool.tile([P, G, NB], f32)
                nc.vector.tensor_copy(out=n0_t[:, :, :], in_=prod_t[:, :, :])
                nc.vector.copy_predicated(out=n0_t[:, :, :], mask=bits_t[:, :, :], data=v0)
                nc.vector.tensor_copy(out=n1_t[:, :, :], in_=v1)
                nc.vector.copy_predicated(out=n1_t[:, :, :], mask=bits_t[:, :, :], data=prod_t[:, :, :])
                nc.vector.tensor_copy(out=out_t[:, :, C + 0:C + 2 * NB:2], in_=n0_t[:, :, :])
                nc.vector.tensor_copy(out=out_t[:, :, C + 1:C + 2 * NB:2], in_=n1_t[:, :, :])

                # logp = sum_j ln(bit ? v0 : v1)
                max_t = pool.tile([P, G, NB], f32)
                nc.vector.tensor_copy(out=max_t[:, :, :], in_=v1)
                nc.vector.copy_predicated(out=max_t[:, :, :], mask=bits_t[:, :, :], data=v0)
                ln_t = pool.tile([P, G, NB], f32)
                nc.scalar.activation(
                    out=ln_t[:, :, :], in_=max_t[:, :, :],
                    func=mybir.ActivationFunctionType.Ln)
                lp_t = lpool.tile([P, G], f32)
                nc.vector.tensor_reduce(
                    out=lp_t[:, :], in_=ln_t[:, :, :],
                    axis=mybir.AxisListType.X, op=mybir.AluOpType.add)

                nc.sync.dma_start(out=out_v[:, sl, :], in_=out_t[:, :, :])
                nc.sync.dma_start(out=lp_v[:, sl], in_=lp_t[:, :])
    nc.compile()
    return nc


def make_in_maps(inputs_continuous, inputs_categorical, dequantization_noise,
                 category_factors=None, binary_mask=None,
                 rows: int = R, n_cores: int = N_CORES):
    in_maps = []
    for i in range(n_cores):
        s = slice(i * rows, (i + 1) * rows)
        in_maps.append({
            "inputs_continuous": np.ascontiguousarray(inputs_continuous[s]),
            "inputs_categorical": np.ascontiguousarray(inputs_categorical[s]),
            "dequantization_noise": np.ascontiguousarray(dequantization_noise[s]),
        })
    return in_maps


def kernel(inputs_continuous, inputs_categorical, dequantization_noise,
           category_factors, binary_mask):
    inputs_continuous = np.asarray(inputs_continuous, dtype=np.float32)
    inputs_categorical = np.asarray(inputs_categorical, dtype=np.int32)
    dequantization_noise = np.asarray(dequantization_noise, dtype=np.float32)

    nc = build_nc()
    in_maps = make_in_maps(inputs_continuous, inputs_categorical,
                           dequantization_noise)
    res = run_bass_kernel_spmd(nc, in_maps, core_ids=list(range(N_CORES)))
    outs = [np.asarray(r["out_continuous"]) for r in res.results]
    lps = [np.asarray(r["log_probs"]) for r in res.results]
    out_continuous = np.concatenate(outs, axis=0).astype(np.float32)
    log_probs = np.concatenate(lps, axis=0).astype(np.float32)
    return out_continuous, log_probs


# revision 19
# speedup vs baseline: 365.5442x; 365.5442x over previous
"""Trainium2 Bass kernel for nn_ArgmaxBase_46523085750826.

Reference computation (per row b of a batch B):
  decimal = sum_k category_factors[k] * inputs_categorical[b, k]        (int)
  bit_j   = (decimal & binary_mask[j]) != 0,  binary_mask[j] = 2^(20-j)
  v0_j = noise[b, 2j], v1_j = noise[b, 2j+1]
  out_noise[b, 2j]   = bit_j ? v0_j      : v0_j*v1_j
  out_noise[b, 2j+1] = bit_j ? v0_j*v1_j : v1_j
  out_continuous[b]  = concat(inputs_continuous[b], out_noise[b])       [106]
  log_probs[b]       = sum_j log(bit_j ? v0_j : v1_j)

category_factors = [6^0 .. 6^7] (mixed radix 6) and binary_mask =
[2^20 .. 2^0] are structural constants of the problem (NCAT=6, NB=21)
and are folded into the kernel (Horner encode + per-column shifts).

Pure data parallel across 8 NeuronCores: core i handles rows
[i*B/8, (i+1)*B/8).
"""

import sys
import numpy as np

if "/opt/trn_rl_repo" not in sys.path:
    sys.path.insert(0, "/opt/trn_rl_repo")

from concourse import bacc, bass, mybir, tile
from concourse.bass_utils import run_bass_kernel_spmd

B = 524288
C = 64          # continuous dims
D = 8           # categorical dims
NCAT = 6
NB = 21         # binary dims
NOUT = C + 2 * NB  # 106
N_CORES = 8
R = B // N_CORES   # rows per core = 65536
P = 128            # partitions


def _stt_int(nc, out, in0, imm, in1, op0, op1):
    """scalar_tensor_tensor with an int32 immediate: out = (in0 op0 imm) op1 in1."""
    eng = nc.vector
    return eng.add_instruction(
        mybir.InstTensorScalarPtr(
            name=nc.get_next_instruction_name(),
            is_scalar_tensor_tensor=True,
            op0=op0, op1=op1,
            ins=[eng.lower_ap(in0),
                 mybir.ImmediateValue(dtype=mybir.dt.int32, value=imm),
                 eng.lower_ap(in1)],
            outs=[eng.lower_ap(out)],
        ))


def build_nc(rows: int = R, G: int = 64, bufs: int = 2, reps: int = 1):
    """Build the per-core Bass graph for a shard of `rows` rows.

    Row -> (partition, slot) mapping is blocked: partition p owns rows
    [p*rows/128, (p+1)*rows/128); each tile covers G consecutive slots
    per partition (128*G rows).

    reps > 1 wraps the whole body in an on-device For_i loop that redoes
    the identical work; used only for timing (per-iteration time =
    (t(reps=a) - t(reps=b)) / (a - b), dispatch overhead cancels).
    """
    n_per_p = rows // P
    assert rows % P == 0 and n_per_p % G == 0
    n_tiles = n_per_p // G

    f32 = mybir.dt.float32
    i32 = mybir.dt.int32

    # Bacc (not plain Bass): its compile() runs generate_event_semaphores,
    # which splits multi-sem waits down to the 1-embedded-wait-per-
    # instruction limit of the TRN2 instruction encodings.
    nc = bacc.Bacc()
    cont_d = nc.declare_dram_parameter("inputs_continuous", [rows, C], f32, isOutput=False)
    cat_d = nc.declare_dram_parameter("inputs_categorical", [rows, D], i32, isOutput=False)
    noise_d = nc.declare_dram_parameter("dequantization_noise", [rows, 2 * NB], f32, isOutput=False)
    out_d = nc.declare_dram_parameter("out_continuous", [rows, NOUT], f32, isOutput=True)
    lp_d = nc.declare_dram_parameter("log_probs", [rows], f32, isOutput=True)

    cont_v = cont_d.rearrange("(p n) c -> p n c", p=P)
    cat_v = cat_d.rearrange("(p n) c -> p n c", p=P)
    noise_v = noise_d.rearrange("(p n) c -> p n c", p=P)
    out_v = out_d.rearrange("(p n) c -> p n c", p=P)
    lp_v = lp_d.rearrange("(p n) -> p n", p=P)

    with tile.TileContext(nc) as tc:
        with tc.tile_pool(name="consts", bufs=1) as cpool, \
             tc.tile_pool(name="lp", bufs=max(2, rows // (P * G))) as lpool, \
             tc.tile_pool(name="work", bufs=bufs) as pool:
            # shift amounts per binary column: 20, 19, ..., 0. Built with DVE
            # memsets (not gpsimd iota) so consumers carry no cross-engine
            # wait: TT/STT encodings only fit one embedded sync wait.
            sh_t = cpool.tile([P, NB], i32)
            for j in range(NB):
                nc.vector.memset(sh_t[:, j:j + 1], NB - 1 - j)

            import contextlib
            loop_cm = tc.For_i(0, reps, 1) if reps > 1 else contextlib.nullcontext()
            with loop_cm:
                _emit_tiles(nc, pool, lpool, sh_t, n_tiles, G,
                            cat_v, noise_v, cont_v, out_v, lp_v)
    nc.compile()
    return nc


def _emit_tiles(nc, pool, lpool, sh_t, n_tiles, G, cat_v, noise_v, cont_v, out_v, lp_v):
    f32 = mybir.dt.float32
    i32 = mybir.dt.int32
    if True:
        if True:  # indentation shim (body moved out of build_nc unchanged)
            for t in range(n_tiles):
                sl = slice(t * G, (t + 1) * G)
                cat_t = pool.tile([P, G, D], i32)
                noise_t = pool.tile([P, G, 2 * NB], f32)
                out_t = pool.tile([P, G, NOUT], f32)
                nc.sync.dma_start(out=cat_t[:, :, :], in_=cat_v[:, sl, :])
                nc.sync.dma_start(out=noise_t[:, :, :], in_=noise_v[:, sl, :])
                nc.sync.dma_start(out=out_t[:, :, 0:C], in_=cont_v[:, sl, :])

                # decimal encode via Horner: dec = (..(c7*6 + c6)*6 ..)*6 + c0
                dec_t = pool.tile([P, G], i32)
                nc.vector.tensor_copy(out=dec_t[:, :], in_=cat_t[:, :, D - 1])
                for k in range(D - 2, -1, -1):
                    nc.vector.scalar_tensor_tensor(
                        out=dec_t[:, :], in0=dec_t[:, :], scalar=float(NCAT),
                        in1=cat_t[:, :, k],
                        op0=mybir.AluOpType.mult, op1=mybir.AluOpType.add)

                # bits[p,g,j] = (dec[p,g] >> (20-j)) & 1  -> uint8 0/1
                # scalar_tensor_tensor (TSP encoding) instead of tensor_tensor:
                # the TT encoding only fits one embedded sync wait, which the
                # tile scheduler can exceed here. Integer immediates must be
                # emitted manually (the helper always lowers them as f32).
                shift_t = pool.tile([P, G, NB], i32)
                _stt_int(nc, out=shift_t[:, :, :],
                         in0=dec_t[:, :, None].broadcast_to([P, G, NB]),
                         imm=0,
                         in1=sh_t[:, None, :].broadcast_to([P, G, NB]),
                         op0=mybir.AluOpType.bypass,
                         op1=mybir.AluOpType.logical_shift_right)
                # bits = (shift & 1) `bypass` shift  (in1 unused by bypass)
                bits_t = pool.tile([P, G, NB], i32)
                _stt_int(nc, out=bits_t[:, :, :], in0=shift_t[:, :, :],
                         imm=1, in1=shift_t[:, :, :],
                         op0=mybir.AluOpType.bitwise_and,
                         op1=mybir.AluOpType.bypass)

                v0 = noise_t[:, :, 0:2 * NB:2]
                v1 = noise_t[:, :, 1:2 * NB:2]

                prod_t = pool.tile([P, G, NB], f32)
                nc.vector.scalar_tensor_tensor(
                    out=prod_t[:, :, :], in0=v0, scalar=0.0, in1=v1,
                    op0=mybir.AluOpType.bypass, op1=mybir.AluOpType.mult)

                # n0 = bit ? v0 : prod ; n1 = bit ? prod : v1
                # (computed in contiguous tiles; interleaved into out_t after)
                n0_t = pool.tile([P, G, NB], f32)
                n1_t = pool.tile([P, G, NB], f32)
                nc.vector.tensor_copy(out=n0_t[:, :, :], in_=prod_t[:, :, :])
                nc.vector.copy_predicated(out=n0_t[:, :, :], mask=bits_t[:, :, :], data=v0)
                nc.vector.tensor_copy(out=n1_t[:, :, :], in_=v1)
                nc.vector.copy_predicated(out=n1_t[:, :, :], mask=bits_t[:, :, :], data=prod_t[:, :, :])
                nc.vector.tensor_copy(out=out_t[:, :, C + 0:C + 2 * NB:2], in_=n0_t[:, :, :])
                nc.vector.tensor_copy(out=out_t[:, :, C + 1:C + 2 * NB:2], in_=n1_t[:, :, :])

                # logp = sum_j ln(bit ? v0 : v1)
                max_t = pool.tile([P, G, NB], f32)
                nc.vector.tensor_copy(out=max_t[:, :, :], in_=v1)
                nc.vector.copy_predicated(out=max_t[:, :, :], mask=bits_t[:, :, :], data=v0)
                ln_t = pool.tile([P, G, NB], f32)
                nc.scalar.activation(
                    out=ln_t[:, :, :], in_=max_t[:, :, :],
                    func=mybir.ActivationFunctionType.Ln)
                lp_t = lpool.tile([P, G], f32)
                nc.vector.tensor_reduce(
                    out=lp_t[:, :], in_=ln_t[:, :, :],
                    axis=mybir.AxisListType.X, op=mybir.AluOpType.add)

                nc.sync.dma_start(out=out_v[:, sl, :], in_=out_t[:, :, :])
                nc.sync.dma_start(out=lp_v[:, sl], in_=lp_t[:, :])


def make_in_maps(inputs_continuous, inputs_categorical, dequantization_noise,
                 category_factors=None, binary_mask=None,
                 rows: int = R, n_cores: int = N_CORES):
    in_maps = []
    for i in range(n_cores):
        s = slice(i * rows, (i + 1) * rows)
        in_maps.append({
            "inputs_continuous": np.ascontiguousarray(inputs_continuous[s]),
            "inputs_categorical": np.ascontiguousarray(inputs_categorical[s]),
            "dequantization_noise": np.ascontiguousarray(dequantization_noise[s]),
        })
    return in_maps


def kernel(inputs_continuous, inputs_categorical, dequantization_noise,
           category_factors, binary_mask):
    inputs_continuous = np.asarray(inputs_continuous, dtype=np.float32)
    inputs_categorical = np.asarray(inputs_categorical, dtype=np.int32)
    dequantization_noise = np.asarray(dequantization_noise, dtype=np.float32)

    nc = build_nc()
    in_maps = make_in_maps(inputs_continuous, inputs_categorical,
                           dequantization_noise)
    res = run_bass_kernel_spmd(nc, in_maps, core_ids=list(range(N_CORES)))
    outs = [np.asarray(r["out_continuous"]) for r in res.results]
    lps = [np.asarray(r["log_probs"]) for r in res.results]
    out_continuous = np.concatenate(outs, axis=0).astype(np.float32)
    log_probs = np.concatenate(lps, axis=0).astype(np.float32)
    return out_continuous, log_probs
